# revision 10
# baseline (speedup 1.0000x reference)
"""Trainium2 Bass kernel for nn_LogicDense (difflogic dense layer).

Math (reference):
    w      = softmax(weight, axis=-1)            # [out_dim, 16]
    coeffs = w @ GATE_COEFFS                     # [out_dim, 4] = (c0, ca, cb, cab)
    a      = x[:, indices[0]]                    # [batch, out_dim]
    b      = x[:, indices[1]]
    out    = c0 + ca*a + cb*b + cab*a*b          # [batch, out_dim]

Strategy (8 NeuronCores, tensor-parallel over out_dim):
    - Host transposes x -> x_t [in_dim, batch] (fp16, replicated to all
      cores). Core c owns output rows j in [2048*c, 2048*(c+1)).
    - Per 128-row chunk: one GPSIMD dma_gather pulls the 256 rows
      x_t[idx0[chunk]] ++ x_t[idx1[chunk]] from HBM into SBUF (row i of
      the index list lands on partition i%128, slot i//128).
    - Per-partition coeff scalars give the combine:
         t = cab*b + ca      (DVE tensor_scalar, fp16 4x mode)
         h = cb*b + c0'      (ACT Identity activation, scale/bias APs)
         g = t * a           (DVE tensor_mul)
         g = g + h           (DVE tensor_add, fp16)
    - out is mathematically in [0, 1] (convex combination of gate
      values); the kernel computes g = OSCALE*out + 0.5. For 10 of 16
      chunks ACT converts g to u8 (store = 0.5 MiB); the other 6 store g
      as fp16 directly (1 MiB) — the mix balances DVE (~97us) and ACT
      (~97us) just under the gather window while keeping HBM at ~45
      MiB/core. OSCALE and +0.5 fold into the host-side gate-coeff
      table since softmax weights sum to 1.
    - Softmax+gate-coeff collapse is computed on device (ACT exp + DVE
      reduces, fp32) from the raw weight shard.
    - Stores ride the HWDGE sync queue; gathers own the SWDGE ring
      (measured 372 GB/s dense). Host reassembles the chunk-typed rows,
      transposes back to [batch, out_dim], dequantizes to fp32.
"""

import os
import sys

import numpy as np

sys.path.insert(0, "/opt/trn_rl_repo")

BATCH = 4096
IN_DIM = 8192
OUT_DIM = 16384
N_CORES = 8
J_SHARD = OUT_DIM // N_CORES        # 2048 output rows per core
CHUNK = 128                         # output rows per pipeline iteration
N_CHUNKS = J_SHARD // CHUNK         # 16
NG = 5                              # gather (ab) buffer sets
NT = 2                              # t buffer sets (ts -> mul lifetime)
NH = 3                              # h buffer sets (h -> add lifetime)
NO = 4                              # o (fp16 g) buffer sets
NO8 = 6                             # o8 (u8) buffer sets
CLAG = 2                            # cvt(i) issued ~2 chunks behind add(i)
DVE_PRE = 14                        # DVE preamble (coeff) instruction count

# Chunks whose g tile is stored as raw fp16 (no u8 cvt): chosen so ACT's
# cvt load (10 chunks) matches DVE's op stream under the gather window.
FP16_CHUNKS = frozenset({2, 5, 8, 11, 14, 15})

GATE_COEFFS = np.array([
    [0, 0, 0, 0], [0, 0, 0, 1], [0, 1, 0, -1], [0, 1, 0, 0],
    [0, 0, 1, -1], [0, 0, 1, 0], [0, 1, 1, -2], [0, 1, 1, -1],
    [1, -1, -1, 1], [1, -1, -1, 2], [1, 0, -1, 0], [1, 0, -1, 1],
    [1, -1, 0, 0], [1, -1, 0, 1], [1, 0, 0, -1], [1, 0, 0, 0],
], dtype=np.float32)                # [16 gates, 4 bilinear coeffs]

# Output is mathematically in [0, 1]; the device computes OSCALE*out + 0.5
# and stores u8 (or that fp16 value directly for FP16_CHUNKS). 254 keeps
# the top of the range clear of 255 so fp16 noise cannot wrap.
OSCALE = 254.0

_CACHE = {}
LAST_RESULT = None  # BassKernelResults of the most recent run (for profiling)


def _wrap_idx16(idx_pair):
    """Wrap the per-core [2, J_SHARD] index array into dma_gather's index
    layout. Per 128-row chunk c the kernel issues ONE gather of 256 indices
    (idx0[chunk] ++ idx1[chunk]); index i of that list lives at
    [i%16, 16*c + i//16], and the 16-partition block is replicated across
    all 8 groups of 16 partitions (the Q7 tx/rx cpus read the indices from
    different partition groups)."""
    cols = []
    for c in range(N_CHUNKS):
        merged = np.concatenate([idx_pair[0, c * CHUNK:(c + 1) * CHUNK],
                                 idx_pair[1, c * CHUNK:(c + 1) * CHUNK]])
        cols.append(merged.astype(np.int16).reshape(16, 16).T)  # [16, 16]
    blk = np.concatenate(cols, axis=1)                # [16, 16*N_CHUNKS]
    return np.ascontiguousarray(np.tile(blk, (8, 1)))  # [128, 256]


def _build_program():
    import concourse.bacc as bacc
    import concourse.mybir as mybir
    from concourse.library_config import mlp
    from contextlib import ExitStack

    dt = mybir.dt
    AF = mybir.ActivationFunctionType

    nc = bacc.Bacc("TRN2", target_bir_lowering=False, debug=False)

    xt = nc.dram_tensor("xt", [IN_DIM, BATCH], dt.float16, kind="ExternalInput")
    idx = nc.dram_tensor("idx", [128, 2 * (J_SHARD // 16)], dt.int16,
                         kind="ExternalInput")
    wgt = nc.dram_tensor("wgt", [128, N_CHUNKS * 16], dt.float32,
                         kind="ExternalInput")
    gcr = nc.dram_tensor("gcr", [128, 4 * N_CHUNKS * 16], dt.float32,
                         kind="ExternalInput")
    out8 = nc.dram_tensor("out8", [J_SHARD, BATCH], dt.uint8,
                          kind="ExternalOutput")
    out16 = nc.dram_tensor("out16", [len(FP16_CHUNKS) * CHUNK, BATCH],
                           dt.float16, kind="ExternalOutput")
    # row block of out16 for each fp16 chunk, in chunk order
    f16_slot = {i: n for n, i in enumerate(sorted(FP16_CHUNKS))}

    W16 = N_CHUNKS * 16  # 256: free size of the wrapped weight / exp tiles

    with ExitStack() as ctx:
        sb = lambda name, shape, dty: ctx.enter_context(
            nc.sbuf_tensor(name, shape, dty))
        sb_idx = sb("sb_idx", [128, 2 * (J_SHARD // 16)], dt.int16)
        sb_w = sb("sb_w", [128, W16], dt.float32)
        sb_gc = sb("sb_gc", [128, 4 * W16], dt.float32)
        sb_e = sb("sb_e", [128, W16], dt.float32)
        sb_scr = sb("sb_scr", [128, W16], dt.float32)
        sb_s = sb("sb_s", [128, N_CHUNKS], dt.float32)
        sb_r = sb("sb_r", [128, N_CHUNKS], dt.float32)
        # coeff tile: [:, 16*k + c] = coeff k (0=c0,1=ca,2=cb,3=cab), chunk c
        sb_cc = sb("sb_cc", [128, 4 * N_CHUNKS], dt.float32)
        # gather dst: slot 0 = a rows, slot 1 = b rows
        ab_bufs = [sb(f"ab{k}", [128, 2, BATCH], dt.float16) for k in range(NG)]
        t_bufs = [sb(f"t{k}", [128, BATCH], dt.float16) for k in range(NT)]
        h_bufs = [sb(f"h{k}", [128, BATCH], dt.float16) for k in range(NH)]
        o_bufs = [sb(f"o{k}", [128, BATCH], dt.float16) for k in range(NO)]
        o8_bufs = [sb(f"q{k}", [128, BATCH], dt.uint8) for k in range(NO8)]

        # Emission plans. ACT: h(i) every chunk, plus the u8 cvt for
        # non-FP16 chunks, lagged CLAG behind. DVE: [ts(i), add(i-1),
        # mul(i)] — every same-engine RAW has >=1 op of separation.
        ops_act = []
        for i in range(N_CHUNKS):
            ops_act.append(('h', i))
            j = i - CLAG
            if j >= 0 and j not in FP16_CHUNKS:
                ops_act.append(('cvt', j))
        for j in range(N_CHUNKS - CLAG, N_CHUNKS):
            if j not in FP16_CHUNKS:
                ops_act.append(('cvt', j))
        act_val = {op: n + 1 for n, op in enumerate(ops_act)}

        ops_dve = []
        for i in range(N_CHUNKS):
            ops_dve.append(('ts', i))
            if i > 0:
                ops_dve.append(('add', i - 1))
            ops_dve.append(('mul', i))
        ops_dve.append(('add', N_CHUNKS - 1))
        dve_val = {op: DVE_PRE + n + 1 for n, op in enumerate(ops_dve)}

        with (
            nc.Block() as block,
            nc.semaphore("s_pi") as s_pi,
            nc.semaphore("s_pw") as s_pw,
            nc.semaphore("s_pg") as s_pg,
            nc.semaphore("s_exp") as s_exp,
            nc.semaphore("s_g0") as s_g0,
            nc.semaphore("s_g1") as s_g1,
            nc.semaphore("s_g2") as s_g2,
            nc.semaphore("s_g3") as s_g3,
            nc.semaphore("s_g4") as s_g4,
            nc.semaphore("s_st0") as s_st0,
            nc.semaphore("s_st1") as s_st1,
            nc.semaphore("s_st2") as s_st2,
            nc.semaphore("s_st3") as s_st3,
            nc.semaphore("s_st4") as s_st4,
            nc.semaphore("s_st5") as s_st5,
            nc.semaphore("s_act") as s_act,
            nc.semaphore("s_dve") as s_dve,
        ):
            s_g = [s_g0, s_g1, s_g2, s_g3, s_g4]
            s_st = [s_st0, s_st1, s_st2, s_st3, s_st4, s_st5]

            def cseg(k, i):  # per-partition scalar AP: coeff k, chunk i
                return sb_cc[:, 16 * k + i : 16 * k + i + 1]

            @block.sync
            def _(sync):
                sync.dma_start(sb_idx[:, :], idx[:, :]).then_inc(s_pi, 16)
                sync.dma_start(sb_w[:, :], wgt[:, :]).then_inc(s_pw, 16)
                sync.dma_start(sb_gc[:, :], gcr[:, :]).then_inc(s_pg, 16)
                for i in range(N_CHUNKS):
                    ks = i % NO8
                    if i in FP16_CHUNKS:
                        sync.wait_ge(s_dve, dve_val[('add', i)])
                        r0 = f16_slot[i] * CHUNK
                        sync.dma_start(out16[r0:r0 + CHUNK, :],
                                       o_bufs[i % NO][:, :],
                                       ).then_inc(s_st[ks], 16)
                    else:
                        sync.wait_ge(s_act, act_val[('cvt', i)])
                        sync.dma_start(out8[i * CHUNK:(i + 1) * CHUNK, :],
                                       o8_bufs[ks][:, :],
                                       ).then_inc(s_st[ks], 16)
                for ks in range(NO8):
                    n_st = (N_CHUNKS - 1 - ks) // NO8 + 1
                    sync.wait_ge(s_st[ks], 16 * n_st)

            @block.gpsimd
            def _(gp):
                gp.load_library(mlp)
                nreg = gp.alloc_register("nidx")
                gp.reg_mov(nreg, 2 * CHUNK)
                gp.wait_ge(s_pi, 16)  # idx tile loaded
                for i in range(N_CHUNKS):
                    kg = i % NG
                    if i >= NG:
                        # ab[kg] free once iter i-NG's b/a readers are done:
                        # DVE mul (a; ts read b before it) + ACT h (b).
                        gp.wait_ge(s_dve, dve_val[('mul', i - NG)])
                        gp.wait_ge(s_act, act_val[('h', i - NG)])
                        gp.wait_ge(s_g[kg], 16 * (i // NG))
                    gp.dma_gather(
                        ab_bufs[kg].ap(), xt.ap(),
                        sb_idx[:, 16 * i:16 * i + 16], 2 * CHUNK, nreg, BATCH,
                    ).then_inc(s_g[kg], 16)

            @block.scalar
            def _(sc):
                sc.wait_ge(s_pw, 16)
                sc.activation(sb_e[:, :], sb_w[:, :], AF.Exp).then_inc(s_exp, 1)
                sc.wait_ge(s_dve, DVE_PRE)  # coeff tile ready
                for kind, i in ops_act:
                    if kind == 'h':
                        k = i % NH
                        kg = i % NG
                        sc.wait_ge(s_g[kg], 16 * (i // NG + 1))
                        # h[k] free once DVE add of i-NH completed
                        if i >= NH:
                            sc.wait_ge(s_dve, dve_val[('add', i - NH)])
                        sc.activation(h_bufs[k][:, :], ab_bufs[kg][:, 1, :],
                                      AF.Identity,
                                      bias=cseg(0, i), scale=cseg(2, i),
                                      ).then_inc(s_act, 1)
                    else:  # cvt on ACT: o8 = u8(g)
                        ko, ks = i % NO, i % NO8
                        sc.wait_ge(s_dve, dve_val[('add', i)])
                        if i >= NO8:
                            sc.wait_ge(s_st[ks], 16 * (i // NO8))
                        sc.activation(o8_bufs[ks][:, :], o_bufs[ko][:, :],
                                      AF.Identity,
                                      ).then_inc(s_act, 1)

            @block.vector
            def _(v):
                # The DVE pipeline is deep: every same-engine RAW below is
                # chained through s_dve (each op incs by 1, dependents wait).
                X = mybir.AxisListType.X
                n = 0

                def step(ins):
                    nonlocal n
                    n += 1
                    ins.then_inc(s_dve, 1)

                v.wait_ge(s_exp, 1)
                v.wait_ge(s_pg, 16)  # gc tile loaded
                e3 = sb_e[:, :].rearrange("p (c g) -> p c g", g=16)
                step(v.reduce_sum(sb_s[:, :], e3, axis=X))
                v.wait_ge(s_dve, n)
                step(v.reciprocal(sb_r[:, :], sb_s[:, :]))
                for kk in range(4):
                    if kk > 0:
                        v.wait_ge(s_dve, n)  # scr free (prior reduce read it)
                    step(v.tensor_mul(sb_scr[:, :], sb_e[:, :],
                                      sb_gc[:, kk * W16:(kk + 1) * W16]))
                    v.wait_ge(s_dve, n)
                    step(v.reduce_sum(
                        sb_cc[:, 16 * kk:16 * (kk + 1)],
                        sb_scr[:, :].rearrange("p (c g) -> p c g", g=16),
                        axis=X))
                v.wait_ge(s_dve, n)  # all cc segments + r landed
                for kk in range(4):
                    step(v.tensor_mul(sb_cc[:, 16 * kk:16 * (kk + 1)],
                                      sb_cc[:, 16 * kk:16 * (kk + 1)],
                                      sb_r[:, :]))
                assert n == DVE_PRE
                v.wait_ge(s_dve, DVE_PRE)  # cc normalize landed
                MU, AD = mybir.AluOpType.mult, mybir.AluOpType.add
                for kind, i in ops_dve:
                    kt, kh, ko, kg = i % NT, i % NH, i % NO, i % NG
                    if kind == 'ts':
                        # t = (b * cab) + ca  (fp16 tensor_scalar, 4x mode)
                        v.wait_ge(s_g[kg], 16 * (i // NG + 1))
                        if i >= NT:
                            # t[kt] free once mul of iter i-NT consumed it
                            v.wait_ge(s_dve, dve_val[('mul', i - NT)])
                        v.tensor_scalar(t_bufs[kt][:, :], ab_bufs[kg][:, 1, :],
                                        cseg(3, i), cseg(1, i), MU, AD,
                                        ).then_inc(s_dve, 1)
                    elif kind == 'mul':
                        v.wait_ge(s_dve, dve_val[('ts', i)])
                        if i >= NO:
                            # o[ko] free once its last reader of iter i-NO is
                            # done: the ACT cvt, or the fp16 store itself.
                            j = i - NO
                            if j in FP16_CHUNKS:
                                v.wait_ge(s_st[j % NO8], 16 * (j // NO8 + 1))
                            else:
                                v.wait_ge(s_act, act_val[('cvt', j)])
                        v.tensor_mul(o_bufs[ko][:, :], t_bufs[kt][:, :],
                                     ab_bufs[kg][:, 0, :]).then_inc(s_dve, 1)
                    else:  # add
                        v.wait_ge(s_act, act_val[('h', i)])
                        v.wait_ge(s_dve, dve_val[('mul', i)])
                        v.tensor_add(o_bufs[ko][:, :], o_bufs[ko][:, :],
                                     h_bufs[kh][:, :]).then_inc(s_dve, 1)

    nc.compile()
    return nc


def _get_program():
    if "nc" not in _CACHE:
        _CACHE["nc"] = _build_program()
    return _CACHE["nc"]


def kernel(x, weight, indices):
    global LAST_RESULT
    from concourse.bass_utils import run_bass_kernel_spmd

    x = np.asarray(x, dtype=np.float32)
    weight = np.asarray(weight, dtype=np.float32)
    indices = np.asarray(indices)

    nc = _get_program()

    xt = np.ascontiguousarray(x.T.astype(np.float16))    # [in_dim, batch]

    # The kernel computes g = OSCALE*out + 0.5; host inverts. Since the
    # softmax weights sum to 1, both the scale and the +0.5 bias fold into
    # the replicated gate-coefficient table: coeffs' = w @ G' with
    # G'[:, 0] = OSCALE*G[:, 0] + 0.5 and G'[:, 1:] = OSCALE*G[:, 1:].
    gate_scaled = GATE_COEFFS * OSCALE
    gate_scaled[:, 0] += 0.5
    # gc replicate: [p, kk*256 + 16*c + g] = gate_scaled[g, kk]
    gc_rep = np.broadcast_to(
        gate_scaled.T.reshape(4, 1, 16),                 # [kk, 1, g]
        (4, N_CHUNKS, 16)).reshape(1, -1)
    gc_rep = np.ascontiguousarray(
        np.broadcast_to(gc_rep, (128, 4 * N_CHUNKS * 16)).astype(np.float32))

    in_maps = []
    for c in range(N_CORES):
        j0 = c * J_SHARD
        idx_c = _wrap_idx16(indices[:, j0:j0 + J_SHARD])
        wsh = weight[j0:j0 + J_SHARD]                    # [2048, 16]
        w_wrapped = np.ascontiguousarray(
            wsh.reshape(N_CHUNKS, 128, 16).transpose(1, 0, 2)
            .reshape(128, N_CHUNKS * 16))
        in_maps.append({
            "xt": xt,
            "idx": idx_c,
            "wgt": w_wrapped,
            "gcr": gc_rep,
        })

    trace = bool(os.environ.get("KERNEL_TRACE"))
    res = run_bass_kernel_spmd(nc, in_maps, core_ids=list(range(N_CORES)),
                               trace=trace)
    LAST_RESULT = res

    f16_rows = sorted(FP16_CHUNKS)
    inv = np.float32(1.0 / OSCALE)
    shards = []
    for c in range(N_CORES):
        g8 = res.results[c]["out8"]                      # [J_SHARD, BATCH] u8
        g16 = res.results[c]["out16"]                    # [6*CHUNK, BATCH] f16
        full = g8.astype(np.float32) * inv
        for n, i in enumerate(f16_rows):
            blk = g16[n * CHUNK:(n + 1) * CHUNK, :].astype(np.float32)
            full[i * CHUNK:(i + 1) * CHUNK, :] = (blk - 0.5) * inv
        shards.append(full)
    full = np.concatenate(shards, axis=0)                # [out_dim, batch]
    return np.ascontiguousarray(full.T)


# revision 13
# speedup vs baseline: 1.0666x; 1.0666x over previous
"""Trainium2 Bass kernel for nn_LogicDense (difflogic dense layer).

Math (reference):
    w      = softmax(weight, axis=-1)            # [out_dim, 16]
    coeffs = w @ GATE_COEFFS                     # [out_dim, 4] = (c0, ca, cb, cab)
    a      = x[:, indices[0]]                    # [batch, out_dim]
    b      = x[:, indices[1]]
    out    = c0 + ca*a + cb*b + cab*a*b          # [batch, out_dim]

Strategy (8 NeuronCores, tensor-parallel over out_dim):
    - Host transposes x -> x_t [in_dim, batch] (fp16, replicated). Core c
      owns output rows j in [2048*c, 2048*(c+1)).
    - Per 128-row chunk: one GPSIMD dma_gather pulls the 256 rows
      x_t[idx0[chunk]] ++ x_t[idx1[chunk]] from HBM into SBUF. Gathers
      alternate between two SWDGE queues.
    - Per-partition coeff scalars give the combine:
         t = cab*b + ca      (DVE tensor_scalar, fp16 4x mode)
         h = cb*b + c0'      (ACT Identity; or DVE tensor_scalar for
                              H_DVE_CHUNKS to offload ACT)
         g = t * a           (DVE tensor_mul)
         g = g + h           (DVE tensor_add, fp16)
    - out is mathematically in [0, 1]; the kernel computes
      g = OSCALE*out + 0.5. For u8 chunks ACT converts g to u8 (0.5 MiB
      store); FP16_CHUNKS write g straight to dedicated store buffers
      (1 MiB) with no conversion op. The mix balances DVE/ACT under the
      gather window. Stores never gate the o-buffer pipeline: fp16 adds
      write dedicated of16 buffers whose reuse distance is ~9 chunks.
    - Same-engine RAWs on DVE rely on in-order execution (no semaphore);
      s_dve only counts mul/add, which cross-engine consumers wait on.
    - Stores ride the HWDGE sync queue; host reassembles the chunk-typed
      rows, transposes to [batch, out_dim], dequantizes to fp32.
"""

import os
import sys

import numpy as np

sys.path.insert(0, "/opt/trn_rl_repo")

BATCH = 4096
IN_DIM = 8192
OUT_DIM = 16384
N_CORES = 8
J_SHARD = OUT_DIM // N_CORES        # 2048 output rows per core
CHUNK = 128                         # output rows per pipeline iteration
N_CHUNKS = J_SHARD // CHUNK         # 16
NG = 4                              # gather (ab) buffer sets
NT = 2                              # t buffer sets
NH = 3                              # h buffer sets
NO = 4                              # o (fp16 g) buffer sets, u8 chunks only
NO8 = 4                             # o8 (u8) buffer sets
NOF = 3                             # of16 (fp16 store) buffer sets
CLAG = 2                            # cvt(i) issued ~2 chunks behind add(i)
DVE_PRE = 14                        # DVE preamble (coeff) instruction count

# Chunks whose g tile is stored as raw fp16 (no cvt op at all).
FP16_CHUNKS = (2, 5, 8, 11, 14, 15)
# Chunks whose h is computed on DVE (tensor_scalar) instead of ACT.
H_DVE_CHUNKS = frozenset({4, 10, 15})

U8_CHUNKS = tuple(i for i in range(N_CHUNKS) if i not in FP16_CHUNKS)
f16_slot = {i: n for n, i in enumerate(FP16_CHUNKS)}     # of16 row block
o8_slot = {i: i % NO8 for i in U8_CHUNKS}
# prior store count on each o8 slot before chunk j stores (for reuse waits)
_o8_prior = {}
for j in U8_CHUNKS:
    _o8_prior[j] = sum(1 for j2 in U8_CHUNKS if j2 < j and j2 % NO8 == j % NO8)
GATE_COEFFS = np.array([
    [0, 0, 0, 0], [0, 0, 0, 1], [0, 1, 0, -1], [0, 1, 0, 0],
    [0, 0, 1, -1], [0, 0, 1, 0], [0, 1, 1, -2], [0, 1, 1, -1],
    [1, -1, -1, 1], [1, -1, -1, 2], [1, 0, -1, 0], [1, 0, -1, 1],
    [1, -1, 0, 0], [1, -1, 0, 1], [1, 0, 0, -1], [1, 0, 0, 0],
], dtype=np.float32)                # [16 gates, 4 bilinear coeffs]

OSCALE = 254.0

_CACHE = {}
LAST_RESULT = None


def _wrap_idx16(idx_pair):
    """Wrap the per-core [2, J_SHARD] index array into dma_gather's index
    layout: per chunk c, index i of (idx0[chunk] ++ idx1[chunk]) lives at
    [i%16, 16*c + i//16], replicated across the 8 groups of 16
    partitions."""
    cols = []
    for c in range(N_CHUNKS):
        merged = np.concatenate([idx_pair[0, c * CHUNK:(c + 1) * CHUNK],
                                 idx_pair[1, c * CHUNK:(c + 1) * CHUNK]])
        cols.append(merged.astype(np.int16).reshape(16, 16).T)  # [16, 16]
    blk = np.concatenate(cols, axis=1)                # [16, 16*N_CHUNKS]
    return np.ascontiguousarray(np.tile(blk, (8, 1)))  # [128, 256]


def _build_program():
    import concourse.bacc as bacc
    import concourse.mybir as mybir
    from concourse.library_config import mlp
    from contextlib import ExitStack

    dt = mybir.dt
    AF = mybir.ActivationFunctionType

    nc = bacc.Bacc("TRN2", target_bir_lowering=False, debug=False,
                   num_swdge_queues=2)

    xt = nc.dram_tensor("xt", [IN_DIM, BATCH], dt.float16, kind="ExternalInput")
    idx = nc.dram_tensor("idx", [128, 2 * (J_SHARD // 16)], dt.int16,
                         kind="ExternalInput")
    wgt = nc.dram_tensor("wgt", [128, N_CHUNKS * 16], dt.float32,
                         kind="ExternalInput")
    gcr = nc.dram_tensor("gcr", [128, 4 * N_CHUNKS * 16], dt.float32,
                         kind="ExternalInput")
    out8 = nc.dram_tensor("out8", [J_SHARD, BATCH], dt.uint8,
                          kind="ExternalOutput")
    out16 = nc.dram_tensor("out16", [len(FP16_CHUNKS) * CHUNK, BATCH],
                           dt.float16, kind="ExternalOutput")

    W16 = N_CHUNKS * 16  # 256: free size of the wrapped weight / exp tiles

    with ExitStack() as ctx:
        sb = lambda name, shape, dty: ctx.enter_context(
            nc.sbuf_tensor(name, shape, dty))
        sb_idx = sb("sb_idx", [128, 2 * (J_SHARD // 16)], dt.int16)
        sb_w = sb("sb_w", [128, W16], dt.float32)
        sb_gc = sb("sb_gc", [128, 4 * W16], dt.float32)
        sb_e = sb("sb_e", [128, W16], dt.float32)
        sb_scr = sb("sb_scr", [128, W16], dt.float32)
        sb_s = sb("sb_s", [128, N_CHUNKS], dt.float32)
        sb_r = sb("sb_r", [128, N_CHUNKS], dt.float32)
        sb_cc = sb("sb_cc", [128, 4 * N_CHUNKS], dt.float32)
        # gather dst: slot 0 = a rows, slot 1 = b rows
        ab_bufs = [sb(f"ab{k}", [128, 2, BATCH], dt.float16) for k in range(NG)]
        t_bufs = [sb(f"t{k}", [128, BATCH], dt.float16) for k in range(NT)]
        h_bufs = [sb(f"h{k}", [128, BATCH], dt.float16) for k in range(NH)]
        o_bufs = [sb(f"o{k}", [128, BATCH], dt.float16) for k in range(NO)]
        o8_bufs = [sb(f"q{k}", [128, BATCH], dt.uint8) for k in range(NO8)]
        of_bufs = [sb(f"f{k}", [128, BATCH], dt.float16) for k in range(NOF)]

        # ACT plan: h(i) for non-H_DVE chunks; u8 cvt(j) lagged CLAG.
        ops_act = []
        for i in range(N_CHUNKS):
            if i not in H_DVE_CHUNKS:
                ops_act.append(('h', i))
            j = i - CLAG
            if j >= 0 and j in U8_CHUNKS:
                ops_act.append(('cvt', j))
        for j in range(N_CHUNKS - CLAG, N_CHUNKS):
            if j in U8_CHUNKS:
                ops_act.append(('cvt', j))
        act_val = {op: n + 1 for n, op in enumerate(ops_act)}

        # s_dve counts preamble + mul/add only (ts/h' have no cross waiters)
        dve_val = {}
        _n = DVE_PRE
        for i in range(N_CHUNKS):
            if i > 0:
                _n += 1
                dve_val[('add', i - 1)] = _n
            _n += 1
            dve_val[('mul', i)] = _n
        _n += 1
        dve_val[('add', N_CHUNKS - 1)] = _n

        with (
            nc.Block() as block,
            nc.semaphore("s_pi") as s_pi,
            nc.semaphore("s_pw") as s_pw,
            nc.semaphore("s_pg") as s_pg,
            nc.semaphore("s_exp") as s_exp,
            nc.semaphore("s_g0") as s_g0,
            nc.semaphore("s_g1") as s_g1,
            nc.semaphore("s_g2") as s_g2,
            nc.semaphore("s_g3") as s_g3,
            nc.semaphore("s_st0") as s_st0,
            nc.semaphore("s_st1") as s_st1,
            nc.semaphore("s_st2") as s_st2,
            nc.semaphore("s_st3") as s_st3,
            nc.semaphore("s_sf0") as s_sf0,
            nc.semaphore("s_sf1") as s_sf1,
            nc.semaphore("s_sf2") as s_sf2,
            nc.semaphore("s_act") as s_act,
            nc.semaphore("s_dve") as s_dve,
        ):
            s_g = [s_g0, s_g1, s_g2, s_g3]
            s_st = [s_st0, s_st1, s_st2, s_st3]
            s_sf = [s_sf0, s_sf1, s_sf2]

            def cseg(k, i):  # per-partition scalar AP: coeff k, chunk i
                return sb_cc[:, 16 * k + i : 16 * k + i + 1]

            @block.sync
            def _(sync):
                sync.dma_start(sb_idx[:, :], idx[:, :]).then_inc(s_pi, 16)
                sync.dma_start(sb_w[:, :], wgt[:, :]).then_inc(s_pw, 16)
                sync.dma_start(sb_gc[:, :], gcr[:, :]).then_inc(s_pg, 16)
                for i in range(N_CHUNKS):
                    if i in f16_slot:
                        sync.wait_ge(s_dve, dve_val[('add', i)])
                        r0 = f16_slot[i] * CHUNK
                        sync.dma_start(out16[r0:r0 + CHUNK, :],
                                       of_bufs[f16_slot[i] % NOF][:, :],
                                       ).then_inc(s_sf[f16_slot[i] % NOF], 16)
                    else:
                        sync.wait_ge(s_act, act_val[('cvt', i)])
                        sync.dma_start(out8[i * CHUNK:(i + 1) * CHUNK, :],
                                       o8_bufs[o8_slot[i]][:, :],
                                       ).then_inc(s_st[o8_slot[i]], 16)
                for ks in range(NO8):
                    n_st = sum(1 for j in U8_CHUNKS if j % NO8 == ks)
                    sync.wait_ge(s_st[ks], 16 * n_st)
                for kf in range(NOF):
                    n_st = sum(1 for n in range(len(FP16_CHUNKS))
                               if n % NOF == kf)
                    sync.wait_ge(s_sf[kf], 16 * n_st)

            @block.gpsimd
            def _(gp):
                gp.load_library(mlp)
                nreg = gp.alloc_register("nidx")
                gp.reg_mov(nreg, 2 * CHUNK)
                gp.wait_ge(s_pi, 16)  # idx tile loaded
                for i in range(N_CHUNKS):
                    kg = i % NG
                    if i >= NG:
                        # ab[kg] free once iter i-NG's readers are done: the
                        # DVE mul comes after ts/ts-h' in-order, so s_dve
                        # mul(i-NG) covers b; ACT h only if it ran there.
                        gp.wait_ge(s_dve, dve_val[('mul', i - NG)])
                        if (i - NG) not in H_DVE_CHUNKS:
                            gp.wait_ge(s_act, act_val[('h', i - NG)])
                        gp.wait_ge(s_g[kg], 16 * (i // NG))
                    gp.dma_gather(
                        ab_bufs[kg].ap(), xt.ap(),
                        sb_idx[:, 16 * i:16 * i + 16], 2 * CHUNK, nreg, BATCH,
                        queue_num=i % 2,
                    ).then_inc(s_g[kg], 16)

            @block.scalar
            def _(sc):
                sc.wait_ge(s_pw, 16)
                sc.activation(sb_e[:, :], sb_w[:, :], AF.Exp).then_inc(s_exp, 1)
                sc.wait_ge(s_dve, DVE_PRE)  # coeff tile ready
                for kind, i in ops_act:
                    if kind == 'h':
                        k = i % NH
                        kg = i % NG
                        sc.wait_ge(s_g[kg], 16 * (i // NG + 1))
                        # h[k] free once DVE add of i-NH completed
                        if i >= NH:
                            sc.wait_ge(s_dve, dve_val[('add', i - NH)])
                        sc.activation(h_bufs[k][:, :], ab_bufs[kg][:, 1, :],
                                      AF.Identity,
                                      bias=cseg(0, i), scale=cseg(2, i),
                                      ).then_inc(s_act, 1)
                    else:  # cvt on ACT: o8 = u8(g)
                        ks = o8_slot[i]
                        sc.wait_ge(s_dve, dve_val[('add', i)])
                        if _o8_prior[i] > 0:
                            sc.wait_ge(s_st[ks], 16 * _o8_prior[i])
                        sc.activation(o8_bufs[ks][:, :], o_bufs[i % NO][:, :],
                                      AF.Identity,
                                      ).then_inc(s_act, 1)

            @block.vector
            def _(v):
                X = mybir.AxisListType.X
                n = 0

                def step(ins):
                    nonlocal n
                    n += 1
                    ins.then_inc(s_dve, 1)

                v.wait_ge(s_exp, 1)
                v.wait_ge(s_pg, 16)  # gc tile loaded
                e3 = sb_e[:, :].rearrange("p (c g) -> p c g", g=16)
                step(v.reduce_sum(sb_s[:, :], e3, axis=X))
                v.wait_ge(s_dve, n)
                step(v.reciprocal(sb_r[:, :], sb_s[:, :]))
                for kk in range(4):
                    if kk > 0:
                        v.wait_ge(s_dve, n)  # scr free (prior reduce read it)
                    step(v.tensor_mul(sb_scr[:, :], sb_e[:, :],
                                      sb_gc[:, kk * W16:(kk + 1) * W16]))
                    v.wait_ge(s_dve, n)
                    step(v.reduce_sum(
                        sb_cc[:, 16 * kk:16 * (kk + 1)],
                        sb_scr[:, :].rearrange("p (c g) -> p c g", g=16),
                        axis=X))
                v.wait_ge(s_dve, n)  # all cc segments + r landed
                for kk in range(4):
                    step(v.tensor_mul(sb_cc[:, 16 * kk:16 * (kk + 1)],
                                      sb_cc[:, 16 * kk:16 * (kk + 1)],
                                      sb_r[:, :]))
                assert n == DVE_PRE
                v.wait_ge(s_dve, DVE_PRE)  # cc normalize landed
                MU, AD = mybir.AluOpType.mult, mybir.AluOpType.add

                def emit_add(i):
                    # g = w + h; fp16 chunks write the dedicated store buf.
                    if i not in H_DVE_CHUNKS:
                        v.wait_ge(s_act, act_val[('h', i)])
                    if i in f16_slot:
                        kf = f16_slot[i] % NOF
                        if f16_slot[i] >= NOF:
                            v.wait_ge(s_sf[kf], 16 * (f16_slot[i] // NOF))
                        dst = of_bufs[kf]
                    else:
                        dst = o_bufs[i % NO]
                    v.tensor_add(dst[:, :], o_bufs[i % NO][:, :],
                                 h_bufs[i % NH][:, :]).then_inc(s_dve, 1)

                for i in range(N_CHUNKS):
                    kt, kg = i % NT, i % NG
                    # t = (b * cab) + ca  (fp16 tensor_scalar, 4x mode).
                    # No then_inc: only same-engine ops consume t/h'.
                    v.wait_ge(s_g[kg], 16 * (i // NG + 1))
                    v.tensor_scalar(t_bufs[kt][:, :], ab_bufs[kg][:, 1, :],
                                    cseg(3, i), cseg(1, i), MU, AD)
                    if i in H_DVE_CHUNKS:
                        # h' = (b * cb) + c0' on DVE (no cross waiter)
                        v.tensor_scalar(h_bufs[i % NH][:, :],
                                        ab_bufs[kg][:, 1, :],
                                        cseg(2, i), cseg(0, i), MU, AD)
                    if i > 0:
                        emit_add(i - 1)
                    # o[i%NO] was last written by chunk i-NO's mul; if that
                    # chunk is u8, its ACT cvt is the last reader. (fp16
                    # chunks' add reads it same-engine, in-order.)
                    if i >= NO and (i - NO) in U8_CHUNKS:
                        v.wait_ge(s_act, act_val[('cvt', i - NO)])
                    v.tensor_mul(o_bufs[i % NO][:, :], t_bufs[kt][:, :],
                                 ab_bufs[kg][:, 0, :]).then_inc(s_dve, 1)
                emit_add(N_CHUNKS - 1)

    nc.compile()
    return nc


def _get_program():
    if "nc" not in _CACHE:
        _CACHE["nc"] = _build_program()
    return _CACHE["nc"]


def kernel(x, weight, indices):
    global LAST_RESULT
    from concourse.bass_utils import run_bass_kernel_spmd

    x = np.asarray(x, dtype=np.float32)
    weight = np.asarray(weight, dtype=np.float32)
    indices = np.asarray(indices)

    nc = _get_program()

    xt = np.ascontiguousarray(x.T.astype(np.float16))    # [in_dim, batch]

    # coeffs' = w @ G' with the OSCALE and +0.5 rounding bias folded in
    # (softmax weights sum to 1).
    gate_scaled = GATE_COEFFS * OSCALE
    gate_scaled[:, 0] += 0.5
    gc_rep = np.broadcast_to(
        gate_scaled.T.reshape(4, 1, 16),                 # [kk, 1, g]
        (4, N_CHUNKS, 16)).reshape(1, -1)
    gc_rep = np.ascontiguousarray(
        np.broadcast_to(gc_rep, (128, 4 * N_CHUNKS * 16)).astype(np.float32))

    in_maps = []
    for c in range(N_CORES):
        j0 = c * J_SHARD
        idx_c = _wrap_idx16(indices[:, j0:j0 + J_SHARD])
        wsh = weight[j0:j0 + J_SHARD]                    # [2048, 16]
        w_wrapped = np.ascontiguousarray(
            wsh.reshape(N_CHUNKS, 128, 16).transpose(1, 0, 2)
            .reshape(128, N_CHUNKS * 16))
        in_maps.append({
            "xt": xt,
            "idx": idx_c,
            "wgt": w_wrapped,
            "gcr": gc_rep,
        })

    trace = bool(os.environ.get("KERNEL_TRACE"))
    res = run_bass_kernel_spmd(nc, in_maps, core_ids=list(range(N_CORES)),
                               trace=trace)
    LAST_RESULT = res

    inv = np.float32(1.0 / OSCALE)
    shards = []
    for c in range(N_CORES):
        g8 = res.results[c]["out8"]                      # [J_SHARD, BATCH] u8
        g16 = res.results[c]["out16"]                    # [6*CHUNK, BATCH] f16
        full = g8.astype(np.float32) * inv
        for n, i in enumerate(FP16_CHUNKS):
            blk = g16[n * CHUNK:(n + 1) * CHUNK, :].astype(np.float32)
            full[i * CHUNK:(i + 1) * CHUNK, :] = (blk - 0.5) * inv
        shards.append(full)
    full = np.concatenate(shards, axis=0)                # [out_dim, batch]
    return np.ascontiguousarray(full.T)
